# revision 23
# baseline (speedup 1.0000x reference)
"""BoxFilter kernel for Trainium2 (8 NeuronCores).

Computes out[b,0,i,j] = sum_{c} sum_{|di|<=15} sum_{|dj|<=15} x[b,c,i+di,j+dj]
(edge-clamped 31x31 box filter over the channel-summed image), matching the
reference cumsum + shifted-diff formulation exactly (separable box sums).

Sharding: data-parallel over (batch, H-half) -> 8 shards, no cross-core
communication. Each core receives a host-padded bf16 [3, 1056, 2048] slab
(16 halo rows on each side, zero-filled past the global image edges).

The problem is HBM-bandwidth-bound; everything on the wire is bf16
(tolerance is 2e-2 relative, bf16 end-to-end lands ~5e-3):
  1. one batched HWDGE DMA per 128-row tile: xc[p, c, n] (1.5 MB)
  2. vertical 31-tap box sum AND the channel sum together on PE: per PSUM
     bank, 6 banded bf16 matmuls accumulate
     sum_c band_a.T @ xc_lo[:,c] + sum_c band_b.T @ xc_hi[:,c].
     PE streams one 512-col matmul per ~215ns (LDWEIGHTS pipelines ahead),
     so 24 matmuls/tile ~ 5.2us; this keeps the vector engine free for
     scans, which are the cadence gate. (GpSimd elementwise is unusable:
     concurrent GpSimd TT and DVE scans slow each other ~2x via SBUF port
     contention.) The 32-row tail tile contracts K=32 only.
  3. ACT copies PSUM (f32) -> zero-padded SBUF tile (bf16)
  4. horizontal 31-tap box sum in one tensor_tensor_scan per row tile
     (state_j = state_{j-1} + xp[j] - xp[j-31]; fp32 internal state)
  5. DMA result rows to DRAM as bf16; host upcasts
"""

import numpy as np
import ml_dtypes

R = 15
TAP = 2 * R + 1          # 31
B, C, H, W = 4, 3, 2048, 2048
HALF = H // 2            # 1024 output rows per core
S_ROWS = HALF + 32       # 1056 input rows per core (16-row halo each side)
N_CORES = 8
PAD_L = TAP              # left zero pad for the scan (31)
PAD_R = R                # right zero pad (15)
XP_W = PAD_L + W + PAD_R # 2094
SCAN_N = W + R           # 2063 scan steps; out col j = scan[j + R]
P = 128                  # SBUF partitions
N_OUT_TILES = HALF // P  # 8
TAIL_ROWS = S_ROWS - N_OUT_TILES * P  # 32 valid rows in the 9th s-tile
MM_N = 512               # one PSUM bank (512 fp32)



_CACHE = {}


def _band_matrices():
    # out row i of a 128-row tile needs halo'd input rows r = i+1 .. i+31
    # (r is the row index within the [s_lo; s_hi] 256-row window).
    k = np.arange(P)[:, None]
    i = np.arange(P)[None, :]
    band_a = ((k >= i + 1) & (k <= i + TAP)).astype(ml_dtypes.bfloat16)
    band_b = ((k + P >= i + 1) & (k + P <= i + TAP)).astype(ml_dtypes.bfloat16)
    return band_a, band_b


def _build_kernel(tc, nc, out, xs, band_a_d, band_b_d, mybir, bass):
    from contextlib import ExitStack

    f32 = mybir.dt.float32
    bf16 = mybir.dt.bfloat16
    add = mybir.AluOpType.add
    sub = mybir.AluOpType.subtract

    with ExitStack() as ctx:
        const_pool = ctx.enter_context(tc.tile_pool(name="const", bufs=1))
        xc_pool = ctx.enter_context(tc.tile_pool(name="xc", bufs=9))
        xp_pool = ctx.enter_context(tc.tile_pool(name="xp", bufs=1))
        box_pool = ctx.enter_context(tc.tile_pool(name="box", bufs=1))
        psum_pool = ctx.enter_context(
            tc.tile_pool(name="psum", bufs=8, space=bass.MemorySpace.PSUM)
        )

        # bands via SWDGE so the sync HWDGE ring's FIFO starts with xc0
        band_a = const_pool.tile([P, P], bf16)
        band_b = const_pool.tile([P, P], bf16)
        nc.gpsimd.dma_start(band_a[:], band_a_d)
        nc.gpsimd.dma_start(band_b[:], band_b_d)

        # persistent rotating xp/box tiles: the zero pads of xp are written
        # once and stay valid (each iteration only overwrites the middle
        # [PAD_L, PAD_L+W) region). 4 box buffers so a slow store never
        # blocks a later scan (WAR), 3 xp buffers likewise for ACT copies.
        N_XP, N_BOX = 3, 4
        xp_tiles = []
        box_tiles = []
        for i in range(N_XP):
            xp = xp_pool.tile([P, XP_W], bf16, tag=f"xp{i}", name=f"xp{i}")
            nc.gpsimd.memset(xp[:, 0:PAD_L], 0.0)
            nc.gpsimd.memset(xp[:, PAD_L + W : XP_W], 0.0)
            xp_tiles.append(xp)
        for i in range(N_BOX):
            box = box_pool.tile([P, SCAN_N + 1], bf16, tag=f"box{i}", name=f"box{i}")
            box_tiles.append(box)

        def make_s(u):
            # one batched DMA for all 3 channels; ALL input loads go FIFO on
            # the sync HWDGE ring (a lone ring gets the full SDMA array --
            # splitting the fill-critical first tiles across both rings
            # dilutes each to a fraction of HBM rate). Stores use scalar.
            rows = P if u < N_OUT_TILES else TAIL_ROWS
            xc = xc_pool.tile([rows, C, W], bf16, tag="xc")
            if u < 2:
                # fill-critical tiles land as 4 column-chunk DMAs so the
                # first matmuls pipeline with the arriving chunks
                for nb in range(W // MM_N):
                    cs = slice(MM_N * nb, MM_N * (nb + 1))
                    nc.sync.dma_start(
                        xc[:rows, :, cs],
                        xs[:, P * u : P * u + rows, cs].rearrange("c p n -> p c n"),
                    )
            else:
                nc.sync.dma_start(
                    xc[:rows],
                    xs[:, P * u : P * u + rows, :].rearrange("c p n -> p c n"),
                )
            return xc

        s_tiles = {0: make_s(0)}
        for t in range(N_OUT_TILES):
            s_tiles[t + 1] = make_s(t + 1)
            xc_lo, xc_hi = s_tiles.pop(t), s_tiles[t + 1]
            hi_k = P if t + 1 < N_OUT_TILES else TAIL_ROWS

            xp = xp_tiles[t % N_XP]
            box = box_tiles[t % N_BOX]

            # vertical box + channel sum in PSUM (f32): 6 bf16 matmuls/bank
            psums = []
            for nb in range(W // MM_N):
                cs = slice(MM_N * nb, MM_N * (nb + 1))
                ps = psum_pool.tile([P, MM_N], f32, tag="ps")
                for c in range(C):
                    nc.tensor.matmul(
                        ps[:], band_a[:], xc_lo[:, c, cs],
                        start=(c == 0), stop=False,
                    )
                psums.append(ps)
            for nb in range(W // MM_N):
                cs = slice(MM_N * nb, MM_N * (nb + 1))
                for c in range(C):
                    nc.tensor.matmul(
                        psums[nb][:], band_b[:hi_k, :], xc_hi[:hi_k, c, cs],
                        start=False, stop=(c == C - 1),
                    )
                nc.scalar.copy(xp[:, PAD_L + MM_N * nb : PAD_L + MM_N * (nb + 1)],
                               psums[nb][:])

            # scan output shifted +1 element so the stored slice starts at a
            # 32B-aligned SBUF offset (misaligned store src halves M2S rate);
            # stores dispatch from sync so ring backpressure never stalls ACT.
            nc.vector.tensor_tensor_scan(
                box[:, 1 : 1 + SCAN_N],
                xp[:, PAD_L : PAD_L + SCAN_N],
                xp[:, 0:SCAN_N],
                0.0,
                add,
                sub,
            )
            nc.sync.dma_start(out[P * t : P * (t + 1), :], box[:, 1 + R : 1 + R + W])


def _get_nc():
    if "nc" in _CACHE:
        return _CACHE["nc"]
    import concourse.bass as bass
    import concourse.tile as tile
    from concourse import bacc, mybir

    nc = bacc.Bacc(
        "TRN2", target_bir_lowering=False, debug=False, num_devices=N_CORES
    )
    xs = nc.dram_tensor("xs", [C, S_ROWS, W], mybir.dt.bfloat16, kind="ExternalInput")
    ba = nc.dram_tensor("band_a", [P, P], mybir.dt.bfloat16, kind="ExternalInput")
    bb = nc.dram_tensor("band_b", [P, P], mybir.dt.bfloat16, kind="ExternalInput")
    out = nc.dram_tensor("out", [HALF, W], mybir.dt.bfloat16, kind="ExternalOutput")

    with tile.TileContext(nc) as tc:
        _build_kernel(tc, nc, out.ap(), xs.ap(), ba.ap(), bb.ap(), mybir, bass)
    nc.compile()
    _CACHE["nc"] = nc
    return nc


def _in_maps(x):
    band_a, band_b = _band_matrices()
    xb = x.astype(ml_dtypes.bfloat16)
    maps = []
    for k in range(N_CORES):
        b, half = divmod(k, 2)
        h0 = half * HALF
        lo = h0 - 16  # global row of xs row 0
        g0, g1 = max(lo, 0), min(h0 + HALF + 16, H)
        xs = np.zeros((C, S_ROWS, W), ml_dtypes.bfloat16)
        xs[:, g0 - lo : g1 - lo, :] = xb[b, :, g0:g1, :]
        maps.append({"xs": xs, "band_a": band_a, "band_b": band_b})
    return maps


def _run(x, trace=False, tmpdir=None):
    from concourse.bass_utils import run_bass_kernel_spmd

    nc = _get_nc()
    res = run_bass_kernel_spmd(
        nc, _in_maps(x), list(range(N_CORES)), trace=trace, tmpdir=tmpdir
    )
    out = np.empty((B, 1, H, W), np.float32)
    for k in range(N_CORES):
        b, half = divmod(k, 2)
        out[b, 0, half * HALF : (half + 1) * HALF, :] = np.asarray(
            res.results[k]["out"]
        ).astype(np.float32)
    return out, res


def kernel(x: np.ndarray) -> np.ndarray:
    x = np.ascontiguousarray(x, dtype=np.float32)
    assert x.shape == (B, C, H, W)
    return _run(x)[0]


# revision 24
# speedup vs baseline: 1.1273x; 1.1273x over previous
"""BoxFilter kernel for Trainium2 (8 NeuronCores).

Computes out[b,0,i,j] = sum_{c} sum_{|di|<=15} sum_{|dj|<=15} x[b,c,i+di,j+dj]
(edge-clamped 31x31 box filter over the channel-summed image), matching the
reference cumsum + shifted-diff formulation exactly (separable box sums).

Sharding: data-parallel over (batch, H-half) -> 8 shards, no cross-core
communication. Each core receives a host-padded bf16 [3, 1056, 2048] slab
(16 halo rows on each side, zero-filled past the global image edges).

The problem is HBM-bandwidth-bound; everything on the wire is bf16
(tolerance is 2e-2 relative, bf16 end-to-end lands ~5e-3):
  1. one batched HWDGE DMA per 128-row tile: xc[p, c, n] (1.5 MB)
  2. vertical 31-tap box sum AND the channel sum together on PE: per PSUM
     bank, 6 banded bf16 matmuls accumulate
     sum_c band_a.T @ xc_lo[:,c] + sum_c band_b.T @ xc_hi[:,c].
     PE streams one 512-col matmul per ~215ns (LDWEIGHTS pipelines ahead),
     so 24 matmuls/tile ~ 5.2us; this keeps the vector engine free for
     scans, which are the cadence gate. (GpSimd elementwise is unusable:
     concurrent GpSimd TT and DVE scans slow each other ~2x via SBUF port
     contention.) The 32-row tail tile contracts K=32 only.
  3. ACT copies PSUM (f32) -> zero-padded SBUF tile (bf16)
  4. horizontal 31-tap box sum in one tensor_tensor_scan per row tile
     (state_j = state_{j-1} + xp[j] - xp[j-31]; fp32 internal state)
  5. DMA result rows to DRAM as bf16; host upcasts
"""

import numpy as np
import ml_dtypes

R = 15
TAP = 2 * R + 1          # 31
B, C, H, W = 4, 3, 2048, 2048
HALF = H // 2            # 1024 output rows per core
S_ROWS = HALF + 32       # 1056 input rows per core (16-row halo each side)
N_CORES = 8
PAD_L = TAP              # left zero pad for the scan (31)
PAD_R = R                # right zero pad (15)
XP_W = PAD_L + W + PAD_R # 2094
SCAN_N = W + R           # 2063 scan steps; out col j = scan[j + R]
P = 128                  # SBUF partitions
N_OUT_TILES = HALF // P  # 8
TAIL_ROWS = S_ROWS - N_OUT_TILES * P  # 32 valid rows in the 9th s-tile
MM_N = 512               # one PSUM bank (512 fp32)



_CACHE = {}


def _band_matrices():
    # out row i of a 128-row tile needs halo'd input rows r = i+1 .. i+31
    # (r is the row index within the [s_lo; s_hi] 256-row window).
    k = np.arange(P)[:, None]
    i = np.arange(P)[None, :]
    band_a = ((k >= i + 1) & (k <= i + TAP)).astype(ml_dtypes.bfloat16)
    band_b = ((k + P >= i + 1) & (k + P <= i + TAP)).astype(ml_dtypes.bfloat16)
    return band_a, band_b


def _build_kernel(tc, nc, out, xs, band_a_d, band_b_d, mybir, bass):
    from contextlib import ExitStack

    f32 = mybir.dt.float32
    bf16 = mybir.dt.bfloat16
    add = mybir.AluOpType.add
    sub = mybir.AluOpType.subtract

    with ExitStack() as ctx:
        const_pool = ctx.enter_context(tc.tile_pool(name="const", bufs=1))
        xc_pool = ctx.enter_context(tc.tile_pool(name="xc", bufs=9))
        xp_pool = ctx.enter_context(tc.tile_pool(name="xp", bufs=1))
        box_pool = ctx.enter_context(tc.tile_pool(name="box", bufs=1))
        psum_pool = ctx.enter_context(
            tc.tile_pool(name="psum", bufs=8, space=bass.MemorySpace.PSUM)
        )

        # bands via SWDGE so the sync HWDGE ring's FIFO starts with xc0
        band_a = const_pool.tile([P, P], bf16)
        band_b = const_pool.tile([P, P], bf16)
        nc.gpsimd.dma_start(band_a[:], band_a_d)
        nc.gpsimd.dma_start(band_b[:], band_b_d)

        # persistent rotating xp/box tiles: the zero pads of xp are written
        # once and stay valid (each iteration only overwrites the middle
        # [PAD_L, PAD_L+W) region). 4 box buffers so a slow store never
        # blocks a later scan (WAR), 3 xp buffers likewise for ACT copies.
        N_XP, N_BOX = 3, 4
        xp_tiles = []
        box_tiles = []
        for i in range(N_XP):
            xp = xp_pool.tile([P, XP_W], bf16, tag=f"xp{i}", name=f"xp{i}")
            nc.gpsimd.memset(xp[:, 0:PAD_L], 0.0)
            nc.gpsimd.memset(xp[:, PAD_L + W : XP_W], 0.0)
            xp_tiles.append(xp)
        for i in range(N_BOX):
            box = box_pool.tile([P, SCAN_N + 1], bf16, tag=f"box{i}", name=f"box{i}")
            box_tiles.append(box)

        def make_s(u):
            # one batched DMA for all 3 channels; ALL input loads go FIFO on
            # the sync HWDGE ring (a lone ring gets the full SDMA array --
            # splitting the fill-critical first tiles across both rings
            # dilutes each to a fraction of HBM rate). Stores use scalar.
            rows = P if u < N_OUT_TILES else TAIL_ROWS
            xc = xc_pool.tile([rows, C, W], bf16, tag="xc")
            if u < 2:
                # fill-critical tiles land as 4 column-chunk DMAs so the
                # first matmuls pipeline with the arriving chunks
                for nb in range(W // MM_N):
                    cs = slice(MM_N * nb, MM_N * (nb + 1))
                    nc.sync.dma_start(
                        xc[:rows, :, cs],
                        xs[:, P * u : P * u + rows, cs].rearrange("c p n -> p c n"),
                    )
            else:
                nc.sync.dma_start(
                    xc[:rows],
                    xs[:, P * u : P * u + rows, :].rearrange("c p n -> p c n"),
                )
            return xc

        s_tiles = {0: make_s(0)}
        for t in range(N_OUT_TILES):
            s_tiles[t + 1] = make_s(t + 1)
            xc_lo, xc_hi = s_tiles.pop(t), s_tiles[t + 1]
            hi_k = P if t + 1 < N_OUT_TILES else TAIL_ROWS

            xp = xp_tiles[t % N_XP]
            box = box_tiles[t % N_BOX]

            # vertical box + channel sum in PSUM (f32): 6 bf16 matmuls/bank
            psums = []
            for nb in range(W // MM_N):
                cs = slice(MM_N * nb, MM_N * (nb + 1))
                ps = psum_pool.tile([P, MM_N], f32, tag="ps")
                for c in range(C):
                    nc.tensor.matmul(
                        ps[:], band_a[:], xc_lo[:, c, cs],
                        start=(c == 0), stop=False,
                    )
                psums.append(ps)
            for nb in range(W // MM_N):
                cs = slice(MM_N * nb, MM_N * (nb + 1))
                for c in range(C):
                    nc.tensor.matmul(
                        psums[nb][:], band_b[:hi_k, :], xc_hi[:hi_k, c, cs],
                        start=False, stop=(c == C - 1),
                    )
                nc.scalar.copy(xp[:, PAD_L + MM_N * nb : PAD_L + MM_N * (nb + 1)],
                               psums[nb][:])

            # scan output shifted +1 element so the stored slice starts at a
            # 32B-aligned SBUF offset (misaligned store src halves M2S rate);
            # stores dispatch from sync so ring backpressure never stalls ACT.
            nc.vector.tensor_tensor_scan(
                box[:, 1 : 1 + SCAN_N],
                xp[:, PAD_L : PAD_L + SCAN_N],
                xp[:, 0:SCAN_N],
                0.0,
                add,
                sub,
            )
            nc.scalar.dma_start(out[P * t : P * (t + 1), :], box[:, 1 + R : 1 + R + W])


def _get_nc():
    if "nc" in _CACHE:
        return _CACHE["nc"]
    import concourse.bass as bass
    import concourse.tile as tile
    from concourse import bacc, mybir

    nc = bacc.Bacc(
        "TRN2", target_bir_lowering=False, debug=False, num_devices=N_CORES
    )
    xs = nc.dram_tensor("xs", [C, S_ROWS, W], mybir.dt.bfloat16, kind="ExternalInput")
    ba = nc.dram_tensor("band_a", [P, P], mybir.dt.bfloat16, kind="ExternalInput")
    bb = nc.dram_tensor("band_b", [P, P], mybir.dt.bfloat16, kind="ExternalInput")
    out = nc.dram_tensor("out", [HALF, W], mybir.dt.bfloat16, kind="ExternalOutput")

    with tile.TileContext(nc) as tc:
        _build_kernel(tc, nc, out.ap(), xs.ap(), ba.ap(), bb.ap(), mybir, bass)
    nc.compile()
    _CACHE["nc"] = nc
    return nc


def _in_maps(x):
    band_a, band_b = _band_matrices()
    xb = x.astype(ml_dtypes.bfloat16)
    maps = []
    for k in range(N_CORES):
        b, half = divmod(k, 2)
        h0 = half * HALF
        lo = h0 - 16  # global row of xs row 0
        g0, g1 = max(lo, 0), min(h0 + HALF + 16, H)
        xs = np.zeros((C, S_ROWS, W), ml_dtypes.bfloat16)
        xs[:, g0 - lo : g1 - lo, :] = xb[b, :, g0:g1, :]
        maps.append({"xs": xs, "band_a": band_a, "band_b": band_b})
    return maps


def _run(x, trace=False, tmpdir=None):
    from concourse.bass_utils import run_bass_kernel_spmd

    nc = _get_nc()
    res = run_bass_kernel_spmd(
        nc, _in_maps(x), list(range(N_CORES)), trace=trace, tmpdir=tmpdir
    )
    out = np.empty((B, 1, H, W), np.float32)
    for k in range(N_CORES):
        b, half = divmod(k, 2)
        out[b, 0, half * HALF : (half + 1) * HALF, :] = np.asarray(
            res.results[k]["out"]
        ).astype(np.float32)
    return out, res


def kernel(x: np.ndarray) -> np.ndarray:
    x = np.ascontiguousarray(x, dtype=np.float32)
    assert x.shape == (B, C, H, W)
    return _run(x)[0]
